# revision 1
# baseline (speedup 1.0000x reference)
"""Trainium2 Bass kernel for the differentiable compressor.

Algorithm
---------
The time recurrence  s_t = a_t s_{t-1} + (1-a_t) v_t,
a_t = A_AT if v_t > s_{t-1} else A_REL  is a max-linear system:
    s_t = max(A_AT s + (1-A_AT) v,  A_REL s + (1-A_REL) v)
so policy iteration converges in a handful of sweeps: guess the modes m_t
from the current trajectory, solve the resulting *linear* recurrence exactly
with the hardware tensor_tensor_scan, repeat.  Empirically (fixed inputs,
key(0)) 4 lagged + 1 exact iteration reach ~1.6e-5 nat ~ 1.4e-4 dB.

Everything runs in natural-log units (v = ln(|x|+1e-8)); the dB scale
cancels through the whole pipeline.  The trajectory is kept relative to the
input, r_t = s_t - v_t, which turns the recurrence into
    r_t = a_t * (r_{t-1} + delta_t),   delta_t = v_{t-1} - v_t,
so the scan is  state = (delta add state) mult a  with delta PRECOMPUTED
once — no per-iteration right-hand-side pass — and the s0 = v init becomes
r0 = 0 (a memset).  Modes are  m_t = [r_{t-1} < -delta_t].

Layout per core: 2 batch rows x 441000 samples -> [126 partitions x 7000],
63 time-chunks per row.  Chunk-boundary carries live in an extra leading
column of the trajectory tile; lagged iterations seed each chunk's scan
with the previous iteration's end-of-previous-chunk state (two tiny
SBUF->SBUF DMAs); the final exact iteration solves chunks with zero
initials, computes true carries via chunk decay products (cumprod scan) +
a [2,63] cross-chunk mini-scan, and distributes them with a fixup pass.
The mode->coefficient affine runs on the Scalar engine in half-width
pieces so it hides under the Vector engine's compare/scan stream.

Sharding: pure data parallel, batch 16 -> 2 rows on each of 8 cores.
"""
import sys
import types
import numpy as np

# ---------------- constants (natural-log units) ----------------
SR = 44100.0
A_AT = float(np.exp(-1.0 / (10.0 * SR / 1000.0)))     # attack coeff
A_REL = float(np.exp(-1.0 / (100.0 * SR / 1000.0)))   # release coeff
DA = A_AT - A_REL
CNAT = float(np.log(10.0) / 20.0)                     # dB -> nat
KN = 0.1 * CNAT                                       # knee
EPS = 1e-4 * CNAT * CNAT
CDN = -(1.0 - 1.0 / 66.7) * 0.5                       # down-ratio gain slope
CUP = (1.0 - 0.1) * 0.5                               # up-ratio gain slope
UPR = 36.0 * CNAT                                     # up-range clamp
TMIN, TMAX = -40.0, 0.0

B, N = 16, 441000
NCORES = 8
ROWS = 2           # batch rows per core
NCH = 63           # chunks per row
P = ROWS * NCH     # 126 partitions
L = N // NCH       # 7000 chunk length
H = L // 2         # half-width for engine overlap

N_LAGGED = 4       # lagged policy iterations before the exact one


def _install_ntff_hook():
    """Inject the missing antenv.axon_hooks so trace=True profiling works."""
    try:
        import antenv
        if "antenv.axon_hooks" not in sys.modules:
            m = types.ModuleType("antenv.axon_hooks")
            m._hook = None
            def _set(h, _m=m): _m._hook = h
            def _get(_m=m): return _m._hook
            m.set_axon_ntff_profile_hook = _set
            m.get_axon_ntff_profile_hook = _get
            sys.modules["antenv.axon_hooks"] = m
            antenv.axon_hooks = m
            from trn_agent_boot.trn_boot import _ntff_profile_via_ctypes
            _set(_ntff_profile_via_ctypes("/opt/axon/libaxon_pjrt.so"))
    except Exception:
        pass


def build_nc():
    import concourse.bacc as bacc
    import concourse.mybir as mybir
    from concourse.tile import TileContext
    from concourse.alu_op_type import AluOpType as Op
    AF = mybir.ActivationFunctionType

    nc = bacc.Bacc("TRN2", target_bir_lowering=False, debug=False)
    x_d = nc.dram_tensor("x", [P, L], mybir.dt.float32, kind="ExternalInput")
    th_d = nc.dram_tensor("th", [P, 1], mybir.dt.float32, kind="ExternalInput")
    dep_d = nc.dram_tensor("dep", [P, 1], mybir.dt.float32, kind="ExternalInput")
    y_d = nc.dram_tensor("y", [P, L], mybir.dt.float32, kind="ExternalOutput")

    f32 = mybir.dt.float32
    with TileContext(nc) as tc:
        with tc.tile_pool(name="pool", bufs=1) as pool:
            tx = pool.tile([P, L], f32)        # x (resident; used at the end)
            tv = pool.tile([P, L], f32)        # v; post: dn-gate scratch
            tD = pool.tile([P, L], f32)        # delta; post: g, m1' scratch
            tse = pool.tile([P, L + 1], f32)   # r trajectory, col0 = boundary
            ta = pool.tile([P, L], f32)        # modes -> a; post: q2 scratch
            tth = pool.tile([P, 1], f32)
            tdep = pool.tile([P, 1], f32)
            tm2 = pool.tile([P, L], f32)       # post: up-gate mask
            te = pool.tile([P, 1], f32)        # exact carries
            tcol = pool.tile([P, 1], f32)      # prev-chunk-end v column
            tG2 = pool.tile([2, NCH], f32)     # transposed chunk decays
            tZ2 = pool.tile([2, NCH], f32)     # transposed chunk end-states
            teb = pool.tile([2, NCH + 1], f32) # mini-scan buffer
            # constant columns for activation bias operands
            tcst = pool.tile([P, 4], f32)
            c1e8, cKN, cEPS, cmKN = (tcst[:, i:i + 1] for i in range(4))
            nc.vector.memset(c1e8, 1e-8)
            nc.vector.memset(cKN, KN)
            nc.vector.memset(cEPS, EPS)
            nc.vector.memset(cmKN, -KN)

            W = tse[:, 1:L + 1]                # trajectory / later w view

            # v = ln(|x|+1e-8), chunked so the x DMA overlaps the ACT chain;
            # delta_t = v_{t-1} - v_t (th cancels within a partition) and
            # -delta.  delta[.,0] crosses chunks via a small column DMA; for
            # each row's first chunk v_{-1} := v_0 so delta = 0 there.
            NS = 8
            CW = L // NS
            nc.sync.dma_start(tth[:], th_d[:])
            nc.sync.dma_start(tdep[:], dep_d[:])
            for j in range(NS):
                sl = slice(j * CW, (j + 1) * CW)
                nc.sync.dma_start(tx[:, sl], x_d[:, sl])
            # r0 = 0 everywhere (s0 = v), including boundary column and the
            # permanent r_{-1}=0 of each row's chunk 0; teb col0 = 0 carry.
            nc.gpsimd.memset(tse[:], 0.0)
            nc.gpsimd.memset(teb[:, 0:1], 0.0)
            for j in range(NS):
                sl = slice(j * CW, (j + 1) * CW)
                nc.scalar.activation(tv[:, sl], tx[:, sl], AF.Abs, bias=0.0, scale=1.0)
                nc.scalar.activation(tv[:, sl], tv[:, sl], AF.Ln, bias=c1e8, scale=1.0)
                lo = j * CW
                s_in = slice(lo if j else 1, (j + 1) * CW)
                s_sh = slice((lo - 1) if j else 0, (j + 1) * CW - 1)
                nc.vector.tensor_tensor(tD[:, s_in], tv[:, s_sh], tv[:, s_in],
                                        Op.subtract)
            nc.sync.dma_start(tcol[1:NCH, 0:1], tv[0:NCH - 1, L - 1:L])
            nc.sync.dma_start(tcol[NCH + 1:P, 0:1], tv[NCH:P - 1, L - 1:L])
            nc.sync.dma_start(tcol[0:1, 0:1], tv[0:1, 0:1])
            nc.sync.dma_start(tcol[NCH:NCH + 1, 0:1], tv[NCH:NCH + 1, 0:1])
            nc.vector.tensor_tensor(tD[:, 0:1], tcol[:, 0:1], tv[:, 0:1], Op.subtract)

            for it in range(N_LAGGED + 1):
                final = it == N_LAGGED
                # modes: m_t = [r_{t-1} < -delta_t].  Column 0 fully split
                # out (its own tiny mode+coeff ops) so the big ops never
                # wait on the column-0 dependency chain (boundary DMA /
                # delta column).  Iteration 0 compares against r==0, a
                # cheaper tensor_scalar.  The a = m*DA + A_REL affine runs
                # on the Scalar engine, hidden under the m/scan stream.
                if it == 0:
                    nc.vector.tensor_scalar(ta[:, 1:H], tD[:, 1:H], 0.0, None,
                                            op0=Op.is_lt)
                else:
                    nc.vector.scalar_tensor_tensor(
                        ta[:, 1:H], tse[:, 1:H], -1.0, tD[:, 1:H],
                        op0=Op.mult, op1=Op.is_gt)
                nc.scalar.activation(ta[:, 1:H], ta[:, 1:H], AF.Copy,
                                     bias=A_REL, scale=DA)
                nc.vector.scalar_tensor_tensor(
                    ta[:, 0:1], tse[:, 0:1], -1.0, tD[:, 0:1],
                    op0=Op.mult, op1=Op.is_gt)
                nc.scalar.activation(ta[:, 0:1], ta[:, 0:1], AF.Copy,
                                     bias=A_REL, scale=DA)
                if it == 0:
                    nc.vector.tensor_scalar(ta[:, H:L], tD[:, H:L], 0.0, None,
                                            op0=Op.is_lt)
                else:
                    nc.vector.scalar_tensor_tensor(
                        ta[:, H:L], tse[:, H:L], -1.0, tD[:, H:L],
                        op0=Op.mult, op1=Op.is_gt)
                nc.scalar.activation(ta[:, H:L], ta[:, H:L], AF.Copy,
                                     bias=A_REL, scale=DA)

                if not final:
                    # lagged carry: initial = previous iterate's boundary col
                    nc.vector.tensor_tensor_scan(
                        tse[:, 1:H + 1], tD[:, 0:H], ta[:, 0:H], tse[:, 0:1],
                        op0=Op.add, op1=Op.mult)
                    nc.vector.tensor_tensor_scan(
                        tse[:, H + 1:L + 1], tD[:, H:L], ta[:, H:L], tse[:, H:H + 1],
                        op0=Op.add, op1=Op.mult)
                    # refresh boundary column from the new trajectory
                    nc.sync.dma_start(tse[1:NCH, 0:1], tse[0:NCH - 1, L:L + 1])
                    nc.sync.dma_start(tse[NCH + 1:P, 0:1], tse[NCH:P - 1, L:L + 1])
                else:
                    # exact: zero-init scans -> W holds z (chunk-local solves)
                    nc.vector.tensor_tensor_scan(
                        tse[:, 1:H + 1], tD[:, 0:H], ta[:, 0:H], 0.0,
                        op0=Op.add, op1=Op.mult)
                    nc.vector.tensor_tensor_scan(
                        tse[:, H + 1:L + 1], tD[:, H:L], ta[:, H:L], tse[:, H:H + 1],
                        op0=Op.add, op1=Op.mult)
                    # chunk end-states transfer while the cumprod scan runs
                    nc.sync.dma_start(tZ2[:], tse[:, L:L + 1])
                    # within-chunk decay cumprod g -> tD (delta is consumed);
                    # op1=bypass ignores data1
                    nc.vector.tensor_tensor_scan(
                        tD[:, 0:H], ta[:, 0:H], ta[:, 0:H], 1.0,
                        op0=Op.mult, op1=Op.bypass)
                    nc.vector.tensor_tensor_scan(
                        tD[:, H:L], ta[:, H:L], ta[:, H:L], tD[:, H - 1:H],
                        op0=Op.mult, op1=Op.bypass)
                    nc.sync.dma_start(tG2[:], tD[:, L - 1:L])
                    # w partial: W += v, overlapping the tG2/te DMA
                    # latency around the tiny cross-chunk mini-scan
                    nc.vector.tensor_tensor(tse[:, 1:H + 1], tse[:, 1:H + 1],
                                            tv[:, 0:H], Op.add)
                    # mini-scan: e[0]=0; e[c] = z0end[c-1] + G[c-1]*e[c-1]
                    nc.vector.tensor_tensor_scan(
                        teb[:, 1:NCH + 1], tG2[:], tZ2[:], 0.0,
                        op0=Op.mult, op1=Op.add)
                    # back to [P,1]: carry BEFORE chunk p = teb[., p]
                    nc.sync.dma_start(te[:], teb[:, 0:NCH])
                    nc.vector.tensor_tensor(tse[:, H + 1:L + 1], tse[:, H + 1:L + 1],
                                            tv[:, H:L], Op.add)
                    # fixup: w = z + v + (g*e - th).  Halved: W's first half
                    # finalizes early so the gain post-processing (ACT
                    # square/sqrt chains) starts while DVE finishes h1.
                    nc.vector.tensor_scalar(tD[:], tD[:], te[:, 0:1], tth[:, 0:1],
                                            op0=Op.mult, op1=Op.subtract)
                    for h in range(2):
                        sl = slice(h * H, (h + 1) * H)
                        slW = slice(h * H + 1, (h + 1) * H + 1)
                        nc.vector.tensor_tensor(tse[:, slW], tse[:, slW],
                                                tD[:, sl], Op.add)

            # ---------------- gain computation ----------------
            # W holds w = s - th = diff_dn.  Per half: gate masks with the
            # gain slopes folded in (DVE), q1/q2 square+sqrt chains (ACT,
            # grouped so the function-table switches stay rare), combines
            # (DVE), then the Exp/multiply/store tail pipelines across
            # ACT / DVE / DMA.
            #   m1' = CDN*[w > -KN]   (down gate)
            #   m2' = CUP*[w < KN]    (up gate)
            #   gdn = (q1 + KN + w) * m1',  q1 = sqrt((w-KN)^2 + EPS)
            #   gup = min((q2 + KN - w) * m2', UPR), q2 = sqrt((w+KN)^2+EPS)
            for h in range(2):
                sl = slice(h * H, (h + 1) * H)
                Wh = tse[:, h * H + 1:(h + 1) * H + 1]
                nc.scalar.activation(tD[:, sl], Wh, AF.Square, bias=cmKN, scale=1.0)
                nc.scalar.activation(ta[:, sl], Wh, AF.Square, bias=cKN, scale=1.0)
                nc.scalar.activation(tD[:, sl], tD[:, sl], AF.Sqrt, bias=cEPS, scale=1.0)
                nc.scalar.activation(ta[:, sl], ta[:, sl], AF.Sqrt, bias=cEPS, scale=1.0)
                nc.vector.tensor_scalar(tv[:, sl], Wh, -KN, CDN, op0=Op.is_gt, op1=Op.mult)
                nc.vector.tensor_scalar(tm2[:, sl], Wh, KN, CUP, op0=Op.is_lt, op1=Op.mult)
                nc.vector.scalar_tensor_tensor(
                    tD[:, sl], tD[:, sl], KN, Wh, op0=Op.add, op1=Op.add)
                nc.vector.tensor_tensor(tD[:, sl], tD[:, sl], tv[:, sl], Op.mult)
                nc.vector.scalar_tensor_tensor(
                    ta[:, sl], ta[:, sl], KN, Wh, op0=Op.add, op1=Op.subtract)
                nc.vector.tensor_tensor(ta[:, sl], ta[:, sl], tm2[:, sl], Op.mult)
                nc.vector.tensor_scalar(ta[:, sl], ta[:, sl], UPR, None, op0=Op.min)
                nc.vector.tensor_tensor(tD[:, sl], tD[:, sl], ta[:, sl], Op.add)
                for q in range(2):
                    sq = slice(h * H + q * (H // 2), h * H + (q + 1) * (H // 2))
                    nc.scalar.activation(tD[:, sq], tD[:, sq], AF.Exp,
                                         bias=0.0, scale=tdep[:, 0:1])
                    nc.vector.tensor_tensor(ta[:, sq], tD[:, sq], tx[:, sq], Op.mult)
                    nc.sync.dma_start(y_d[:, sq], ta[:, sq])

    nc.compile()
    return nc


_NC = None


def _get_nc():
    global _NC
    if _NC is None:
        _NC = build_nc()
    return _NC


def make_in_maps(x, threshold, depth):
    th_nat = ((TMIN + threshold.astype(np.float32) * (TMAX - TMIN)) *
              np.float32(CNAT)).astype(np.float32)           # [16,1]
    dep = depth.astype(np.float32)
    in_maps = []
    for i in range(NCORES):
        xs = np.ascontiguousarray(x[ROWS * i:ROWS * (i + 1)]).reshape(P, L)
        ths = np.repeat(th_nat[ROWS * i:ROWS * (i + 1), 0], NCH).reshape(P, 1)
        deps = np.repeat(dep[ROWS * i:ROWS * (i + 1), 0], NCH).reshape(P, 1)
        in_maps.append({"x": xs.astype(np.float32),
                        "th": np.ascontiguousarray(ths, np.float32),
                        "dep": np.ascontiguousarray(deps, np.float32)})
    return in_maps


def kernel(x, threshold, depth):
    _install_ntff_hook()
    from concourse.bass_utils import run_bass_kernel_spmd
    nc = _get_nc()
    x = np.asarray(x, np.float32)
    in_maps = make_in_maps(x, np.asarray(threshold), np.asarray(depth))
    res = run_bass_kernel_spmd(nc, in_maps, core_ids=list(range(NCORES)))
    y = np.empty((B, N), np.float32)
    for i in range(NCORES):
        y[ROWS * i:ROWS * (i + 1)] = np.asarray(res.results[i]["y"]).reshape(ROWS, N)
    return y



# revision 5
# speedup vs baseline: 1.6013x; 1.6013x over previous
"""Trainium2 Bass kernel for the differentiable compressor.

Algorithm
---------
The smoothing recurrence  s_t = a_t s_{t-1} + (1-a_t) v_t,
a_t = A_AT if v_t > s_{t-1} else A_REL  is solved by lagged policy
iteration in relative coordinates r_t = s_t - v_t:
    r_t = a_t * (r_{t-1} + delta_t),   delta_t = v_{t-1} - v_t
with delta precomputed once.  Key identity: since a_t > 0, the next
sweep's mode  m_t = [r_{t-1} + delta_t < 0]  equals  [r_t < 0]  - a
single-tensor sign test of the current trajectory, so each sweep costs
one 2x-mode tensor_scalar + the scan (the carry staleness it adds at
chunk-leading columns is ~1e-4 relative, verified off-line).

Layout per core: 2 batch rows x 441000 samples -> [126 x 7000], 63
time-chunks per row; chunk carries live in an extra leading column of
the trajectory tile, refreshed between sweeps by two tiny SBUF->SBUF
DMAs.  4 sweeps (lagged carries, no exact pass) land at ~7.6e-4
relative error vs the 2e-2 gate.

Gain: the knee's eps-smoothing (width 0.01 dB) is dropped and the
two gates become
    g = min(2CUP relu(-(w+KN)), UPR-cupK) - |2CDN| relu(w-KN)
        - Kbar sign(w) + C0
(exact outside the 0.1 dB knee interior; ~1.7e-4 overall), which runs
the nonlinear pieces on the Scalar engine - Relu/Relu/Sign/Exp plus
Abs/Ln all live in one activation table (explicitly pinned) so there
are zero mid-kernel table loads.  DVE per quarter: min-TS, STT, add,
y-multiply, pipelined against ACT and the output DMA.

Sharding: pure data parallel, batch 16 -> 2 rows on each of 8 cores.
"""
import sys
import types
import numpy as np

# ---------------- constants (natural-log units) ----------------
SR = 44100.0
A_AT = float(np.exp(-1.0 / (10.0 * SR / 1000.0)))     # attack coeff
A_REL = float(np.exp(-1.0 / (100.0 * SR / 1000.0)))   # release coeff
DA = A_AT - A_REL
CNAT = float(np.log(10.0) / 20.0)                     # dB -> nat
KN = 0.1 * CNAT                                       # knee
M2CDN = 1.0 - 1.0 / 66.7                              # |2*CDN|
M2CUP = 1.0 - 0.1                                     # 2*CUP
CDNK = M2CDN * KN
CUPK = M2CUP * KN
UPR = 36.0 * CNAT
UPRP = UPR - CUPK
C0 = (CUPK - CDNK) / 2.0
KBAR = (CUPK + CDNK) / 2.0
TMIN, TMAX = -40.0, 0.0

B, N = 16, 441000
NCORES = 8
ROWS = 2           # batch rows per core
NCH = 63           # chunks per row
P = ROWS * NCH     # 126 partitions
L = N // NCH       # 7000 chunk length
H = L // 2         # half width
Q = L // 4         # quarter width
NS = 8             # x DMA / start-phase chunks
CW = L // NS

N_SWEEPS = 4


def _install_ntff_hook():
    """Inject the missing antenv.axon_hooks so trace=True profiling works."""
    try:
        import antenv
        if "antenv.axon_hooks" not in sys.modules:
            m = types.ModuleType("antenv.axon_hooks")
            m._hook = None
            def _set(h, _m=m): _m._hook = h
            def _get(_m=m): return _m._hook
            m.set_axon_ntff_profile_hook = _set
            m.get_axon_ntff_profile_hook = _get
            sys.modules["antenv.axon_hooks"] = m
            antenv.axon_hooks = m
            from trn_agent_boot.trn_boot import _ntff_profile_via_ctypes
            _set(_ntff_profile_via_ctypes("/opt/axon/libaxon_pjrt.so"))
    except Exception:
        pass


def build_nc():
    import concourse.bacc as bacc
    import concourse.mybir as mybir
    from concourse.tile import TileContext
    from concourse.alu_op_type import AluOpType as Op
    AF = mybir.ActivationFunctionType
    f32 = mybir.dt.float32

    nc = bacc.Bacc("TRN2", target_bir_lowering=False, debug=False)
    x_d = nc.dram_tensor("x", [P, L], f32, kind="ExternalInput")
    nth_d = nc.dram_tensor("nth", [P, 1], f32, kind="ExternalInput")   # -th
    dep_d = nc.dram_tensor("dep", [P, 1], f32, kind="ExternalInput")
    bx_d = nc.dram_tensor("bx", [P, 1], f32, kind="ExternalInput")     # dep*C0
    y_d = nc.dram_tensor("y", [P, L], f32, kind="ExternalOutput")

    with TileContext(nc) as tc:
        with tc.tile_pool(name="pool", bufs=1) as pool:
            tx = pool.tile([P, L], f32)        # x (kept for final multiply)
            tv = pool.tile([P, L], f32)        # v -> v' = v - th; post: A3
            tD = pool.tile([P, L], f32)        # delta; post: A1/u/G/gain
            tse = pool.tile([P, L + 1], f32)   # r trajectory, col0 = carry
            ta = pool.tile([P, L], f32)        # modes -> a; post: A2/p/y
            tnth = pool.tile([P, 1], f32)
            tdep = pool.tile([P, 1], f32)
            tbx = pool.tile([P, 1], f32)
            tvL = pool.tile([P, 1], f32)       # v[:, L-1] (early)
            tcol = pool.tile([P, 1], f32)      # prev-chunk-end v column
            # constant columns for activation bias operands
            tcst = pool.tile([P, 4], f32)
            c1e8, cbup, cbdn, crel = (tcst[:, i:i + 1] for i in range(4))
            nc.vector.memset(c1e8, 1e-8)
            nc.vector.memset(cbup, -M2CUP * KN)
            nc.vector.memset(cbdn, -M2CDN * KN)
            nc.vector.memset(crel, A_REL)

            # pin the activation table that holds abs/ln/identity/relu/sign/exp
            atl = mybir.InstLoadActFuncSet(
                name=nc.get_next_instruction_name(), ins=[], outs=[],
                act_func_set_id=6)
            nc.scalar.add_instruction(atl)

            nc.sync.dma_start(tnth[:], nth_d[:])
            nc.sync.dma_start(tdep[:], dep_d[:])
            nc.sync.dma_start(tbx[:], bx_d[:])
            # last x column first: unblocks the cross-chunk delta column
            nc.sync.dma_start(tvL[:], x_d[:, L - 1:L])
            for j in range(NS):
                sl = slice(j * CW, (j + 1) * CW)
                nc.sync.dma_start(tx[:, sl], x_d[:, sl])
            nc.gpsimd.memset(tse[:, 0:1], 0.0)

            # v[:, L-1] = ln(|x_L-1| + 1e-8), then shift across partitions
            nc.scalar.activation(tvL[:], tvL[:], AF.Abs, bias=0.0, scale=1.0)
            nc.scalar.activation(tvL[:], tvL[:], AF.Ln, bias=c1e8, scale=1.0)
            nc.sync.dma_start(tcol[1:NCH, 0:1], tvL[0:NCH - 1, 0:1])
            nc.sync.dma_start(tcol[NCH + 1:P, 0:1], tvL[NCH:P - 1, 0:1])

            # chunked: v = ln(|x|+1e-8); delta = v_{t-1} - v_t; it-0 modes
            # m0 = [delta < 0] and a0 = A_REL + DA*m0 (affine kept on DVE so
            # the Scalar engine only does abs+ln during the latency-critical
            # start).  Chunk 0 skips its col 0 (cross-chunk; fixed below).
            for j in range(NS):
                sl = slice(j * CW, (j + 1) * CW)
                nc.scalar.activation(tv[:, sl], tx[:, sl], AF.Abs, bias=0.0, scale=1.0)
                nc.scalar.activation(tv[:, sl], tv[:, sl], AF.Ln, bias=c1e8, scale=1.0)
                lo = j * CW
                s_in = slice(lo if j else 1, (j + 1) * CW)
                s_sh = slice((lo - 1) if j else 0, (j + 1) * CW - 1)
                nc.vector.tensor_tensor(tD[:, s_in], tv[:, s_sh], tv[:, s_in],
                                        Op.subtract)
                nc.vector.tensor_scalar(ta[:, s_in], tD[:, s_in], 0.0, DA,
                                        op0=Op.is_lt, op1=Op.mult)
                nc.vector.tensor_scalar(ta[:, s_in], ta[:, s_in], A_REL, None,
                                        op0=Op.add)
            # col-0 fixes: rows 0 and NCH have no predecessor chunk -> delta 0
            nc.sync.dma_start(tcol[0:1, 0:1], tv[0:1, 0:1])
            nc.sync.dma_start(tcol[NCH:NCH + 1, 0:1], tv[NCH:NCH + 1, 0:1])
            nc.vector.tensor_tensor(tD[:, 0:1], tcol[:, 0:1], tv[:, 0:1], Op.subtract)
            nc.vector.tensor_scalar(ta[:, 0:1], tD[:, 0:1], 0.0, DA,
                                    op0=Op.is_lt, op1=Op.mult)
            nc.vector.tensor_scalar(ta[:, 0:1], ta[:, 0:1], A_REL, None,
                                    op0=Op.add)

            # v' = v - th on ACT (hidden under the sweep scans)
            for h in range(2):
                sl = slice(h * H, (h + 1) * H)
                nc.scalar.activation(tv[:, sl], tv[:, sl], AF.Identity,
                                     bias=tnth[:, 0:1], scale=1.0)

            # ---------------- sweeps ----------------
            # DVE order: [scan_k h1][TS_{k+1} h1][scan_k h2][TS_{k+1} h2] ...
            # ACT affines and carry DMAs slot in between via dependencies.
            for k in range(N_SWEEPS):
                last = k == N_SWEEPS - 1
                for h in range(2):
                    sl = slice(h * H, (h + 1) * H)
                    slW = slice(h * H + 1, (h + 1) * H + 1)
                    nc.vector.tensor_tensor_scan(
                        tse[:, slW], tD[:, sl], ta[:, sl],
                        tse[:, h * H:h * H + 1],
                        op0=Op.add, op1=Op.mult)
                    if not last:
                        # next sweep's modes: m = [r < 0]; a = A_REL + DA*m
                        nc.vector.tensor_scalar(ta[:, sl], tse[:, slW], 0.0, DA,
                                                op0=Op.is_lt, op1=Op.mult)
                        nc.scalar.activation(ta[:, sl], ta[:, sl], AF.Identity,
                                             bias=crel, scale=1.0)
                if not last:
                    nc.sync.dma_start(tse[1:NCH, 0:1], tse[0:NCH - 1, L:L + 1])
                    nc.sync.dma_start(tse[NCH + 1:P, 0:1], tse[NCH:P - 1, L:L + 1])
            # W = r + v' in place — after BOTH last-sweep scans (the h2 scan
            # reads col H as its initial; the in-place add must not clobber it)
            for h in range(2):
                sl = slice(h * H, (h + 1) * H)
                slW = slice(h * H + 1, (h + 1) * H + 1)
                nc.vector.tensor_tensor(tse[:, slW], tse[:, slW],
                                        tv[:, sl], Op.add)

            # ---------------- gain ----------------
            #   A1 = relu(-M2CUP*(w+KN));   u = min(A1, UPRP)      (up branch)
            #   A2 = relu( M2CDN*(w-KN))                           (down)
            #   A3 = sign(w);  p = -KBAR*A3 - A2;  G = u + p
            #   gain = exp(dep*G + dep*C0);  y = gain * x
            for q in range(4):
                sl = slice(q * Q, (q + 1) * Q)
                w = tse[:, q * Q + 1:(q + 1) * Q + 1]
                nc.scalar.activation(tD[:, sl], w, AF.Relu,
                                     bias=cbup, scale=-M2CUP)
                nc.scalar.activation(ta[:, sl], w, AF.Relu,
                                     bias=cbdn, scale=M2CDN)
                nc.scalar.activation(tv[:, sl], w, AF.Sign, bias=0.0, scale=1.0)
                nc.vector.tensor_scalar(tD[:, sl], tD[:, sl], UPRP, None,
                                        op0=Op.min)
                nc.vector.scalar_tensor_tensor(
                    ta[:, sl], tv[:, sl], -KBAR, ta[:, sl],
                    op0=Op.mult, op1=Op.subtract)
                nc.vector.tensor_tensor(tD[:, sl], tD[:, sl], ta[:, sl], Op.add)
                nc.scalar.activation(tD[:, sl], tD[:, sl], AF.Exp,
                                     bias=tbx[:, 0:1], scale=tdep[:, 0:1])
                nc.vector.tensor_tensor(ta[:, sl], tD[:, sl], tx[:, sl], Op.mult)
                nc.sync.dma_start(y_d[:, sl], ta[:, sl])

    nc.compile()
    return nc


_NC = None


def _get_nc():
    global _NC
    if _NC is None:
        _NC = build_nc()
    return _NC


def make_in_maps(x, threshold, depth):
    th_nat = ((TMIN + threshold.astype(np.float32) * (TMAX - TMIN)) *
              np.float32(CNAT)).astype(np.float32)           # [16,1]
    dep = depth.astype(np.float32)
    bx = (dep * np.float32(C0)).astype(np.float32)
    in_maps = []
    for i in range(NCORES):
        xs = np.ascontiguousarray(x[ROWS * i:ROWS * (i + 1)]).reshape(P, L)
        nth = np.repeat(-th_nat[ROWS * i:ROWS * (i + 1), 0], NCH).reshape(P, 1)
        deps = np.repeat(dep[ROWS * i:ROWS * (i + 1), 0], NCH).reshape(P, 1)
        bxs = np.repeat(bx[ROWS * i:ROWS * (i + 1), 0], NCH).reshape(P, 1)
        in_maps.append({"x": xs.astype(np.float32),
                        "nth": np.ascontiguousarray(nth, np.float32),
                        "dep": np.ascontiguousarray(deps, np.float32),
                        "bx": np.ascontiguousarray(bxs, np.float32)})
    return in_maps


def kernel(x, threshold, depth):
    _install_ntff_hook()
    from concourse.bass_utils import run_bass_kernel_spmd
    nc = _get_nc()
    x = np.asarray(x, np.float32)
    in_maps = make_in_maps(x, np.asarray(threshold), np.asarray(depth))
    res = run_bass_kernel_spmd(nc, in_maps, core_ids=list(range(NCORES)))
    y = np.empty((B, N), np.float32)
    for i in range(NCORES):
        y[ROWS * i:ROWS * (i + 1)] = np.asarray(res.results[i]["y"]).reshape(ROWS, N)
    return y


# revision 6
# speedup vs baseline: 1.7641x; 1.1017x over previous
"""Trainium2 Bass kernel for the differentiable compressor.

Algorithm
---------
The smoothing recurrence  s_t = a_t s_{t-1} + (1-a_t) v_t,
a_t = A_AT if v_t > s_{t-1} else A_REL  is solved by lagged policy
iteration in relative coordinates r_t = s_t - v_t:
    r_t = a_t * (r_{t-1} + delta_t),   delta_t = v_{t-1} - v_t
with delta precomputed once.  Key identity: since a_t > 0, the next
sweep's mode  m_t = [r_{t-1} + delta_t < 0]  equals  [r_t < 0], so each
sweep's coefficients come from a Sign + affine pair on the Scalar
engine - the Vector engine runs scans back to back.  4 sweeps with
lagged chunk carries land at ~7.6e-4 relative error vs the 2e-2 gate.

Layout per core: 2 batch rows x 441000 samples -> [126 x 7000], 63
time-chunks per row; chunk carries live in an extra leading column of
the trajectory tile, refreshed between sweeps by two tiny SBUF->SBUF
DMAs.

Gain: the knee's eps-smoothing (width 0.01 dB) is dropped and the two
gates collapse to
    g = min(2CUP relu(-(w+KN)), UPR-cupK) - |2CDN| relu(w-KN)
        - Kbar sign(w) + C0,    w = level - th
(exact outside the 0.1 dB knee interior; ~1.7e-4 overall).  The -th
shift rides in the per-partition bias operands of the three Scalar-
engine ops, Relu/Relu/Sign/Exp/Abs/Ln all live in one activation
table (explicitly pinned, zero mid-kernel table loads), and the Vector
engine does two fused scalar_tensor_tensors + the y multiply per
quarter, pipelined against ACT and the output DMA.

Sharding: pure data parallel, batch 16 -> 2 rows on each of 8 cores.
"""
import sys
import types
import numpy as np

# ---------------- constants (natural-log units) ----------------
SR = 44100.0
A_AT = float(np.exp(-1.0 / (10.0 * SR / 1000.0)))     # attack coeff
A_REL = float(np.exp(-1.0 / (100.0 * SR / 1000.0)))   # release coeff
DA = A_AT - A_REL
AMID = (A_AT + A_REL) / 2.0
CNAT = float(np.log(10.0) / 20.0)                     # dB -> nat
KN = 0.1 * CNAT                                       # knee
M2CDN = 1.0 - 1.0 / 66.7                              # |2*CDN|
M2CUP = 1.0 - 0.1                                     # 2*CUP
CDNK = M2CDN * KN
CUPK = M2CUP * KN
UPR = 36.0 * CNAT
UPRP = UPR - CUPK
C0 = (CUPK - CDNK) / 2.0
KBAR = (CUPK + CDNK) / 2.0
TMIN, TMAX = -40.0, 0.0

B, N = 16, 441000
NCORES = 8
ROWS = 2           # batch rows per core
NCH = 63           # chunks per row
P = ROWS * NCH     # 126 partitions
L = N // NCH       # 7000 chunk length
H = L // 2         # half width
Q = L // 4         # quarter width
NS = 8             # x DMA / start-phase chunks
CW = L // NS

N_SWEEPS = 4


def _install_ntff_hook():
    """Inject the missing antenv.axon_hooks so trace=True profiling works."""
    try:
        import antenv
        if "antenv.axon_hooks" not in sys.modules:
            m = types.ModuleType("antenv.axon_hooks")
            m._hook = None
            def _set(h, _m=m): _m._hook = h
            def _get(_m=m): return _m._hook
            m.set_axon_ntff_profile_hook = _set
            m.get_axon_ntff_profile_hook = _get
            sys.modules["antenv.axon_hooks"] = m
            antenv.axon_hooks = m
            from trn_agent_boot.trn_boot import _ntff_profile_via_ctypes
            _set(_ntff_profile_via_ctypes("/opt/axon/libaxon_pjrt.so"))
    except Exception:
        pass


def build_nc():
    import concourse.bacc as bacc
    import concourse.mybir as mybir
    from concourse.tile import TileContext
    from concourse.alu_op_type import AluOpType as Op
    AF = mybir.ActivationFunctionType
    f32 = mybir.dt.float32

    nc = bacc.Bacc("TRN2", target_bir_lowering=False, debug=False)
    x_d = nc.dram_tensor("x", [P, L], f32, kind="ExternalInput")
    # per-partition scalars: [-th, dep, dep*C0, bup, bdn]
    sc_d = nc.dram_tensor("sc", [P, 5], f32, kind="ExternalInput")
    y_d = nc.dram_tensor("y", [P, L], f32, kind="ExternalOutput")

    with TileContext(nc) as tc:
        with tc.tile_pool(name="pool", bufs=1) as pool:
            tx = pool.tile([P, L], f32)        # x (kept for final multiply)
            tv = pool.tile([P, L], f32)        # v; post: A3 scratch
            tD = pool.tile([P, L], f32)        # delta; post: A1/u/G/gain
            tse = pool.tile([P, L + 1], f32)   # r trajectory, col0 = carry
            ta = pool.tile([P, L], f32)        # modes -> a; post: A2/p/y
            tsc = pool.tile([P, 5], f32)
            tvL = pool.tile([P, 1], f32)       # v[:, L-1] (early)
            tcol = pool.tile([P, 1], f32)      # prev-chunk-end v column
            # constant columns for activation bias operands
            tcst = pool.tile([P, 2], f32)
            c1e8, cmid = (tcst[:, i:i + 1] for i in range(2))
            nc.vector.memset(c1e8, 1e-8)
            nc.vector.memset(cmid, AMID)
            nth, dep, bx, bup, bdn = (tsc[:, i:i + 1] for i in range(5))

            # pin the act table holding abs/ln/identity/relu/sign/exp
            atl = mybir.InstLoadActFuncSet(
                name=nc.get_next_instruction_name(), ins=[], outs=[],
                act_func_set_id=6)
            nc.scalar.add_instruction(atl)

            # last x column first: unblocks the cross-chunk delta column
            nc.sync.dma_start(tvL[:], x_d[:, L - 1:L])
            nc.sync.dma_start(tx[:, 0:CW], x_d[:, 0:CW])
            nc.sync.dma_start(tx[:, CW:2 * CW], x_d[:, CW:2 * CW])
            nc.sync.dma_start(tsc[:], sc_d[:])
            for j in range(2, NS):
                sl = slice(j * CW, (j + 1) * CW)
                nc.sync.dma_start(tx[:, sl], x_d[:, sl])
            nc.gpsimd.memset(tse[:, 0:1], 0.0)

            # v[:, L-1] = ln(|x_L-1| + 1e-8), then shift across partitions
            nc.scalar.activation(tvL[:], tvL[:], AF.Abs, bias=0.0, scale=1.0)
            nc.scalar.activation(tvL[:], tvL[:], AF.Ln, bias=c1e8, scale=1.0)
            nc.sync.dma_start(tcol[1:NCH, 0:1], tvL[0:NCH - 1, 0:1])
            nc.sync.dma_start(tcol[NCH + 1:P, 0:1], tvL[NCH:P - 1, 0:1])

            # chunked: v = ln(|x|+1e-8); delta = v_{t-1} - v_t; it-0 modes
            # m0 = [delta < 0], a0 = A_REL + DA*m0 (both on DVE, hidden
            # under the DMA/Ln stream).  Chunk 0's col 0 is cross-chunk:
            # its delta/a ops are emitted right after chunk 0 (the Vector
            # engine runs its queue in order - emitting them any later
            # would gate the first scan on the last chunk).
            for j in range(NS):
                sl = slice(j * CW, (j + 1) * CW)
                nc.scalar.activation(tv[:, sl], tx[:, sl], AF.Abs, bias=0.0, scale=1.0)
                nc.scalar.activation(tv[:, sl], tv[:, sl], AF.Ln, bias=c1e8, scale=1.0)
                lo = j * CW
                s_in = slice(lo if j else 1, (j + 1) * CW)
                s_sh = slice((lo - 1) if j else 0, (j + 1) * CW - 1)
                nc.vector.tensor_tensor(tD[:, s_in], tv[:, s_sh], tv[:, s_in],
                                        Op.subtract)
                nc.vector.tensor_scalar(ta[:, s_in], tD[:, s_in], 0.0, DA,
                                        op0=Op.is_lt, op1=Op.mult)
                nc.vector.tensor_scalar(ta[:, s_in], ta[:, s_in], A_REL, None,
                                        op0=Op.add)
                if j == 0:
                    # col-0: rows 0 and NCH have no predecessor -> delta 0
                    nc.sync.dma_start(tcol[0:1, 0:1], tv[0:1, 0:1])
                    nc.sync.dma_start(tcol[NCH:NCH + 1, 0:1],
                                      tv[NCH:NCH + 1, 0:1])
                    nc.vector.tensor_tensor(tD[:, 0:1], tcol[:, 0:1],
                                            tv[:, 0:1], Op.subtract)
                    nc.vector.tensor_scalar(ta[:, 0:1], tD[:, 0:1], 0.0, DA,
                                            op0=Op.is_lt, op1=Op.mult)
                    nc.vector.tensor_scalar(ta[:, 0:1], ta[:, 0:1], A_REL, None,
                                            op0=Op.add)

            # ---------------- sweeps ----------------
            # DVE runs scans back to back; the next sweep's coefficients
            # a = AMID - (DA/2)*sign(r) come from a Sign+Identity pair on
            # the Scalar engine, hidden under the opposite half's scan.
            for k in range(N_SWEEPS):
                last = k == N_SWEEPS - 1
                for h in range(2):
                    sl = slice(h * H, (h + 1) * H)
                    slW = slice(h * H + 1, (h + 1) * H + 1)
                    nc.vector.tensor_tensor_scan(
                        tse[:, slW], tD[:, sl], ta[:, sl],
                        tse[:, h * H:h * H + 1],
                        op0=Op.add, op1=Op.mult)
                    if not last:
                        nc.scalar.activation(ta[:, sl], tse[:, slW], AF.Sign,
                                             bias=0.0, scale=1.0)
                        nc.scalar.activation(ta[:, sl], ta[:, sl], AF.Identity,
                                             bias=cmid, scale=-DA / 2.0)
                if not last:
                    nc.sync.dma_start(tse[1:NCH, 0:1], tse[0:NCH - 1, L:L + 1])
                    nc.sync.dma_start(tse[NCH + 1:P, 0:1], tse[NCH:P - 1, L:L + 1])

            # W = r + v in place (quartered; after BOTH last-sweep scans -
            # the h2 scan reads col H as its initial state)
            for q in range(4):
                nc.vector.tensor_tensor(tse[:, q * Q + 1:(q + 1) * Q + 1],
                                        tse[:, q * Q + 1:(q + 1) * Q + 1],
                                        tv[:, q * Q:(q + 1) * Q], Op.add)

            # ---------------- gain ----------------
            #   A1 = relu(-M2CUP*W + bup);  A2 = relu(M2CDN*W + bdn)
            #   A3 = sign(W - th);  p = -KBAR*A3 - A2
            #   G = min(UPRP, A1) + p;  gain = exp(dep*G + dep*C0)
            for q in range(4):
                sl = slice(q * Q, (q + 1) * Q)
                w = tse[:, q * Q + 1:(q + 1) * Q + 1]
                nc.scalar.activation(tD[:, sl], w, AF.Relu,
                                     bias=bup, scale=-M2CUP)
                nc.scalar.activation(ta[:, sl], w, AF.Relu,
                                     bias=bdn, scale=M2CDN)
                nc.scalar.activation(tv[:, sl], w, AF.Sign, bias=nth, scale=1.0)
                nc.vector.scalar_tensor_tensor(
                    ta[:, sl], tv[:, sl], -KBAR, ta[:, sl],
                    op0=Op.mult, op1=Op.subtract)
                nc.vector.scalar_tensor_tensor(
                    tD[:, sl], tD[:, sl], UPRP, ta[:, sl],
                    op0=Op.min, op1=Op.add)
                nc.scalar.activation(tD[:, sl], tD[:, sl], AF.Exp,
                                     bias=bx, scale=dep[:, 0:1])
                nc.vector.tensor_tensor(ta[:, sl], tD[:, sl], tx[:, sl], Op.mult)
                nc.sync.dma_start(y_d[:, sl], ta[:, sl])

    nc.compile()
    return nc


_NC = None


def _get_nc():
    global _NC
    if _NC is None:
        _NC = build_nc()
    return _NC


def make_in_maps(x, threshold, depth):
    th_nat = ((TMIN + threshold.astype(np.float32) * (TMAX - TMIN)) *
              np.float32(CNAT)).astype(np.float32)           # [16,1]
    dep = depth.astype(np.float32)
    in_maps = []
    for i in range(NCORES):
        xs = np.ascontiguousarray(x[ROWS * i:ROWS * (i + 1)]).reshape(P, L)
        th_c = np.repeat(th_nat[ROWS * i:ROWS * (i + 1), 0], NCH)    # [P]
        dep_c = np.repeat(dep[ROWS * i:ROWS * (i + 1), 0], NCH)
        sc = np.stack([-th_c,
                       dep_c,
                       dep_c * np.float32(C0),
                       np.float32(M2CUP) * (th_c - np.float32(KN)),
                       np.float32(-M2CDN) * (th_c + np.float32(KN))],
                      axis=1)
        in_maps.append({"x": xs.astype(np.float32),
                        "sc": np.ascontiguousarray(sc, np.float32)})
    return in_maps


def kernel(x, threshold, depth):
    _install_ntff_hook()
    from concourse.bass_utils import run_bass_kernel_spmd
    nc = _get_nc()
    x = np.asarray(x, np.float32)
    in_maps = make_in_maps(x, np.asarray(threshold), np.asarray(depth))
    res = run_bass_kernel_spmd(nc, in_maps, core_ids=list(range(NCORES)))
    y = np.empty((B, N), np.float32)
    for i in range(NCORES):
        y[ROWS * i:ROWS * (i + 1)] = np.asarray(res.results[i]["y"]).reshape(ROWS, N)
    return y


# revision 9
# speedup vs baseline: 1.8241x; 1.0340x over previous
"""Trainium2 Bass kernel for the differentiable compressor.

Algorithm
---------
The smoothing recurrence  s_t = a_t s_{t-1} + (1-a_t) v_t,
a_t = A_AT if v_t > s_{t-1} else A_REL  is solved by lagged policy
iteration in relative coordinates r_t = s_t - v_t:
    r_t = a_t * (r_{t-1} + delta_t),   delta_t = v_{t-1} - v_t
with delta precomputed once.  Key identity: since a_t > 0, the next
sweep's mode  m_t = [r_{t-1} + delta_t < 0]  equals  [r_t < 0], so each
sweep's coefficients come from a Sign + affine pair on the Scalar
engine - the Vector engine runs scans back to back.  4 sweeps with
lagged chunk carries land at ~7.6e-4 relative error vs the 2e-2 gate.

Layout per core: 2 batch rows x 441000 samples -> [126 x 7000], 63
time-chunks per row; chunk carries live in an extra leading column of
the trajectory tile, refreshed between sweeps by two tiny SBUF->SBUF
DMAs.

Gain: the knee's eps-smoothing (width 0.01 dB) is dropped and the two
gates collapse to
    g = min(2CUP relu(-(w+KN)), UPR-cupK) - |2CDN| relu(w-KN)
        - Kbar sign(w) + C0,    w = level - th
(exact outside the 0.1 dB knee interior; ~1.7e-4 overall).  The -th
shift rides in the per-partition bias operands of the three Scalar-
engine ops, Relu/Relu/Sign/Exp/Abs/Ln all live in one activation
table (explicitly pinned, zero mid-kernel table loads), and the Vector
engine does two fused scalar_tensor_tensors + the y multiply per
quarter, pipelined against ACT and the output DMA.

Sharding: pure data parallel, batch 16 -> 2 rows on each of 8 cores.
"""
import sys
import types
import numpy as np

# ---------------- constants (natural-log units) ----------------
SR = 44100.0
A_AT = float(np.exp(-1.0 / (10.0 * SR / 1000.0)))     # attack coeff
A_REL = float(np.exp(-1.0 / (100.0 * SR / 1000.0)))   # release coeff
DA = A_AT - A_REL
AMID = (A_AT + A_REL) / 2.0
CNAT = float(np.log(10.0) / 20.0)                     # dB -> nat
KN = 0.1 * CNAT                                       # knee
M2CDN = 1.0 - 1.0 / 66.7                              # |2*CDN|
M2CUP = 1.0 - 0.1                                     # 2*CUP
CDNK = M2CDN * KN
CUPK = M2CUP * KN
UPR = 36.0 * CNAT
UPRP = UPR - CUPK
C0 = (CUPK - CDNK) / 2.0
KBAR = (CUPK + CDNK) / 2.0
TMIN, TMAX = -40.0, 0.0

B, N = 16, 441000
NCORES = 8
ROWS = 2           # batch rows per core
NCH = 63           # chunks per row
P = ROWS * NCH     # 126 partitions
L = N // NCH       # 7000 chunk length
H = L // 2         # half width
Q = L // 4         # quarter width
NS = 8             # x DMA / start-phase chunks
CW = L // NS

N_SWEEPS = 4


def _install_ntff_hook():
    """Inject the missing antenv.axon_hooks so trace=True profiling works."""
    try:
        import antenv
        if "antenv.axon_hooks" not in sys.modules:
            m = types.ModuleType("antenv.axon_hooks")
            m._hook = None
            def _set(h, _m=m): _m._hook = h
            def _get(_m=m): return _m._hook
            m.set_axon_ntff_profile_hook = _set
            m.get_axon_ntff_profile_hook = _get
            sys.modules["antenv.axon_hooks"] = m
            antenv.axon_hooks = m
            from trn_agent_boot.trn_boot import _ntff_profile_via_ctypes
            _set(_ntff_profile_via_ctypes("/opt/axon/libaxon_pjrt.so"))
    except Exception:
        pass


def build_nc():
    import concourse.bacc as bacc
    import concourse.mybir as mybir
    from concourse.tile import TileContext
    from concourse.alu_op_type import AluOpType as Op
    AF = mybir.ActivationFunctionType
    f32 = mybir.dt.float32

    nc = bacc.Bacc("TRN2", target_bir_lowering=False, debug=False)
    x_d = nc.dram_tensor("x", [P, L], f32, kind="ExternalInput")
    # per-partition scalars: [-th, dep, dep*C0, bup, bdn]
    sc_d = nc.dram_tensor("sc", [P, 5], f32, kind="ExternalInput")
    # carry shift matrix: shm[src, dst] = 1 iff dst == src+1 and dst % NCH != 0
    shm_d = nc.dram_tensor("shm", [P, P], f32, kind="ExternalInput")
    y_d = nc.dram_tensor("y", [P, L], f32, kind="ExternalOutput")

    with TileContext(nc) as tc:
        with tc.tile_pool(name="pool", bufs=1) as pool, \
             tc.psum_pool(name="pp", bufs=1) as pp:
            tx = pool.tile([P, L], f32)        # x (kept for final multiply)
            tv = pool.tile([P, L], f32)        # v; post: A3 scratch
            tD = pool.tile([P, L], f32)        # delta; post: A1/u/G/gain
            tse = pool.tile([P, L], f32)       # r trajectory
            ta = pool.tile([P, L], f32)        # modes -> a; post: A2/p/y
            tsc = pool.tile([P, 5], f32)
            tshm = pool.tile([P, P], f32)      # carry shift matrix
            tcar = pp.tile([P, 1], f32)        # shifted carries (PSUM)
            tvL = pool.tile([P, 1], f32)       # v[:, L-1] (early)
            tcol = pool.tile([P, 1], f32)      # prev-chunk-end v column
            # constant columns for activation bias operands
            tcst = pool.tile([P, 2], f32)
            c1e8, cmid = (tcst[:, i:i + 1] for i in range(2))
            nc.vector.memset(c1e8, 1e-8)
            nc.vector.memset(cmid, AMID)
            nth, dep, bx, bup, bdn = (tsc[:, i:i + 1] for i in range(5))

            # pin the act table holding abs/ln/identity/relu/sign/exp
            atl = mybir.InstLoadActFuncSet(
                name=nc.get_next_instruction_name(), ins=[], outs=[],
                act_func_set_id=6)
            nc.scalar.add_instruction(atl)

            # last x column first: unblocks the cross-chunk delta column
            nc.sync.dma_start(tvL[:], x_d[:, L - 1:L])
            nc.sync.dma_start(tx[:, 0:CW], x_d[:, 0:CW])
            nc.sync.dma_start(tx[:, CW:2 * CW], x_d[:, CW:2 * CW])
            nc.sync.dma_start(tsc[:], sc_d[:])
            nc.sync.dma_start(tshm[:], shm_d[:])
            for j in range(2, NS):
                sl = slice(j * CW, (j + 1) * CW)
                nc.sync.dma_start(tx[:, sl], x_d[:, sl])

            # v[:, L-1] = ln(|x_L-1| + 1e-8), then shift across partitions
            nc.scalar.activation(tvL[:], tvL[:], AF.Abs, bias=0.0, scale=1.0)
            nc.scalar.activation(tvL[:], tvL[:], AF.Ln, bias=c1e8, scale=1.0)
            nc.sync.dma_start(tcol[1:NCH, 0:1], tvL[0:NCH - 1, 0:1])
            nc.sync.dma_start(tcol[NCH + 1:P, 0:1], tvL[NCH:P - 1, 0:1])

            # chunked: v = ln(|x|+1e-8); delta = v_{t-1} - v_t; it-0 modes
            # m0 = [delta < 0], a0 = A_REL + DA*m0 (both on DVE, hidden
            # under the DMA/Ln stream).  Chunk 0's col 0 is cross-chunk:
            # its delta/a ops are emitted right after chunk 0 (the Vector
            # engine runs its queue in order - emitting them any later
            # would gate the first scan on the last chunk).
            for j in range(NS):
                sl = slice(j * CW, (j + 1) * CW)
                nc.scalar.activation(tv[:, sl], tx[:, sl], AF.Abs, bias=0.0, scale=1.0)
                nc.scalar.activation(tv[:, sl], tv[:, sl], AF.Ln, bias=c1e8, scale=1.0)
                lo = j * CW
                s_in = slice(lo if j else 1, (j + 1) * CW)
                s_sh = slice((lo - 1) if j else 0, (j + 1) * CW - 1)
                nc.vector.tensor_tensor(tD[:, s_in], tv[:, s_sh], tv[:, s_in],
                                        Op.subtract)
                nc.vector.tensor_scalar(ta[:, s_in], tD[:, s_in], 0.0, DA,
                                        op0=Op.is_lt, op1=Op.mult)
                nc.vector.tensor_scalar(ta[:, s_in], ta[:, s_in], A_REL, None,
                                        op0=Op.add)
                if j == 0:
                    # col-0: rows 0 and NCH have no predecessor -> delta 0
                    nc.sync.dma_start(tcol[0:1, 0:1], tv[0:1, 0:1])
                    nc.sync.dma_start(tcol[NCH:NCH + 1, 0:1],
                                      tv[NCH:NCH + 1, 0:1])
                    nc.vector.tensor_tensor(tD[:, 0:1], tcol[:, 0:1],
                                            tv[:, 0:1], Op.subtract)
                    nc.vector.tensor_scalar(ta[:, 0:1], tD[:, 0:1], 0.0, DA,
                                            op0=Op.is_lt, op1=Op.mult)
                    nc.vector.tensor_scalar(ta[:, 0:1], ta[:, 0:1], A_REL, None,
                                            op0=Op.add)

            # ---------------- sweeps ----------------
            # DVE runs scans back to back; the next sweep's coefficients
            # a = AMID - (DA/2)*sign(r) come from a Sign+Identity pair on
            # the Scalar engine, hidden under the opposite half's scan.
            # Chunk carries: the Tensor engine multiplies the end column by
            # a shifted-identity matrix (zero rows at chunk-0 positions)
            # into PSUM, which the next h1 scan reads as its initial state
            # - much lower latency than an SBUF->SBUF partition-shift DMA.
            for k in range(N_SWEEPS):
                last = k == N_SWEEPS - 1
                for h in range(2):
                    sl = slice(h * H, (h + 1) * H)
                    init = 0.0 if k == 0 and h == 0 else (
                        tcar[:, 0:1] if h == 0 else tse[:, H - 1:H])
                    nc.vector.tensor_tensor_scan(
                        tse[:, sl], tD[:, sl], ta[:, sl], init,
                        op0=Op.add, op1=Op.mult)
                    if not last:
                        nc.scalar.activation(ta[:, sl], tse[:, sl], AF.Sign,
                                             bias=0.0, scale=1.0)
                        nc.scalar.activation(ta[:, sl], ta[:, sl], AF.Identity,
                                             bias=cmid, scale=-DA / 2.0)
                if not last:
                    nc.tensor.matmul(tcar[:], tshm[:], tse[:, L - 1:L],
                                     start=True, stop=True)

            # ---------------- tail: W = r + v, then gain ----------------
            #   A1 = relu(-M2CUP*W + bup);  A2 = relu(M2CDN*W + bdn)
            #   A3 = sign(W - th);  p = -KBAR*A3 - A2
            #   G = min(UPRP, A1) + p;  gain = exp(dep*G + dep*C0)
            for q in range(4):
                sl = slice(q * Q, (q + 1) * Q)
                w = tse[:, sl]
                nc.vector.tensor_tensor(w, w, tv[:, sl], Op.add)
                nc.scalar.activation(tD[:, sl], w, AF.Relu,
                                     bias=bup, scale=-M2CUP)
                nc.scalar.activation(ta[:, sl], w, AF.Relu,
                                     bias=bdn, scale=M2CDN)
                nc.scalar.activation(tv[:, sl], w, AF.Sign, bias=nth, scale=1.0)
                nc.vector.scalar_tensor_tensor(
                    ta[:, sl], tv[:, sl], -KBAR, ta[:, sl],
                    op0=Op.mult, op1=Op.subtract)
                nc.vector.scalar_tensor_tensor(
                    tD[:, sl], tD[:, sl], UPRP, ta[:, sl],
                    op0=Op.min, op1=Op.add)
                nc.scalar.activation(tD[:, sl], tD[:, sl], AF.Exp,
                                     bias=bx, scale=dep[:, 0:1])
                if q < 3:
                    nc.vector.tensor_tensor(ta[:, sl], tD[:, sl], tx[:, sl], Op.mult)
                    nc.sync.dma_start(y_d[:, sl], ta[:, sl])
                else:
                    for e in range(2):
                        se = slice(q * Q + e * (Q // 2), q * Q + (e + 1) * (Q // 2))
                        nc.vector.tensor_tensor(ta[:, se], tD[:, se], tx[:, se],
                                                Op.mult)
                        nc.sync.dma_start(y_d[:, se], ta[:, se])

    nc.compile()
    return nc


_NC = None


def _get_nc():
    global _NC
    if _NC is None:
        _NC = build_nc()
    return _NC


def make_in_maps(x, threshold, depth):
    th_nat = ((TMIN + threshold.astype(np.float32) * (TMAX - TMIN)) *
              np.float32(CNAT)).astype(np.float32)           # [16,1]
    dep = depth.astype(np.float32)
    # shm[src, dst] = 1 iff dst == src+1 and dst is not a chunk-0 row
    shm = np.zeros((P, P), np.float32)
    for src in range(P - 1):
        dst = src + 1
        if dst % NCH != 0:
            shm[src, dst] = 1.0
    in_maps = []
    for i in range(NCORES):
        xs = np.ascontiguousarray(x[ROWS * i:ROWS * (i + 1)]).reshape(P, L)
        th_c = np.repeat(th_nat[ROWS * i:ROWS * (i + 1), 0], NCH)    # [P]
        dep_c = np.repeat(dep[ROWS * i:ROWS * (i + 1), 0], NCH)
        sc = np.stack([-th_c,
                       dep_c,
                       dep_c * np.float32(C0),
                       np.float32(M2CUP) * (th_c - np.float32(KN)),
                       np.float32(-M2CDN) * (th_c + np.float32(KN))],
                      axis=1)
        in_maps.append({"x": xs.astype(np.float32),
                        "sc": np.ascontiguousarray(sc, np.float32),
                        "shm": shm})
    return in_maps


def kernel(x, threshold, depth):
    _install_ntff_hook()
    from concourse.bass_utils import run_bass_kernel_spmd
    nc = _get_nc()
    x = np.asarray(x, np.float32)
    in_maps = make_in_maps(x, np.asarray(threshold), np.asarray(depth))
    res = run_bass_kernel_spmd(nc, in_maps, core_ids=list(range(NCORES)))
    y = np.empty((B, N), np.float32)
    for i in range(NCORES):
        y[ROWS * i:ROWS * (i + 1)] = np.asarray(res.results[i]["y"]).reshape(ROWS, N)
    return y
